# revision 24
# baseline (speedup 1.0000x reference)
"""Trainium2 Bass kernel for nn_Critic (bidirectional-LSTM critic network).

Data-parallel over the B (sequence) dimension: 8 NeuronCores x 512 sequences.
v2 design notes:

  - fp16 data everywhere on-chip (1 cyc/row PE matmuls, DVE 2x/4x modes);
    psum stays fp32, LN/softmax statistics stay fp32-ish via psum.
  - ALL sigmoids are computed as tanh: sigma(z) = (tanh(z/2)+1)/2, with the
    1/2-scales folded into the gate weights/biases and the (+1, x0.5) affines
    folded into fused DVE scalar_tensor_tensor ops on a x2-scaled cell state
    (ctilde = 2c, htilde = 2h; W_h and the head weights absorb the 0.5).
    Result: every LSTM ACT instr is a plain tanh -> one activation table
    (exp/tanh/square/relu) for LSTM; trunk runs phase-separated on the
    rsqrt table; no per-step ACT table thrash.
  - Trunk LN: mean-centering folded into W (W @ (I-1/64)); g folded into the
    PE rstd-replicate matmul; relu+beta as a single DVE tensor_scalar.
    rsqrt on ACT (the baseline's 3.3us-per-instr DVE RECIPROCAL is gone).
  - LSTM state buffers XH_f/XH_b [128, 65*512]: rows 0:64 = htilde written by
    DVE each step, rows 64:128 = trunk x output (written twice, once per
    direction) -> gate matmuls read one contiguous [128,256] rhs, no per-step
    x staging copies. Gates packed (f|i), (o|j) -> M=128 matmuls, one tanh
    per psum tile. Batch split into 2 chains of 256 seqs to hide the serial
    step latency.
  - Head: softmax via exp + row-sums + a single DVE divide (no reciprocal).
"""

import sys

for _p in ("/opt/trn_rl_repo",):
    if _p not in sys.path:
        sys.path.insert(0, _p)

import json as _json
from types import MethodType as _MethodType

import numpy as np

import concourse.bass as bass
import concourse.tile as tile
from concourse import mybir
from concourse.bass_utils import run_bass_kernel_spmd

F32 = mybir.dt.float32
F16 = mybir.dt.float16
AF = mybir.ActivationFunctionType
ALU = mybir.AluOpType

B, T, H, OBS, ACTD = 4096, 64, 64, 128, 32
NCORES = 8
BC = B // NCORES  # 512 sequences per core
NT = T * BC  # 32768 rows per core
EPS = 1e-12
HB = BC // 2  # 256 per chain

# two-ended t-block order: iter j handles blocks (j, 63-j)
ORD = [t for j in range(32) for t in (j, 63 - j)]

# ---------------------------------------------------------------- waitfix --
# This walrus build rejects instructions carrying more than one sync-wait
# command. The Tile kernel-tail drain (and barriers) routinely carry more.
# Patch the serialized BIR: move excess waits onto inserted NoOp carriers.
_MAX_WAITS = 1


def _patch_bir(bir):
    n = [0]

    def fresh():
        n[0] += 1
        return f"I-waitfix-{n[0]}"

    for fn in bir.get("functions", []):
        for bb in fn.get("blocks", []):
            out = []
            for inst in bb.get("instructions", []):
                si = inst.get("sync_info") or {}
                waits = si.get("on_wait") or []
                if len(waits) > _MAX_WAITS:
                    extra = waits[: len(waits) - _MAX_WAITS]
                    keep = waits[len(waits) - _MAX_WAITS :]
                    for i in range(0, len(extra), _MAX_WAITS):
                        out.append(
                            {
                                "name": fresh(),
                                "opcode": "NoOp",
                                "engine": inst["engine"],
                                "ins": [],
                                "outs": [],
                                "sync_info": {
                                    "on_wait": extra[i : i + _MAX_WAITS],
                                    "on_update": [],
                                },
                            }
                        )
                    si = dict(si)
                    si["on_wait"] = keep
                    inst = dict(inst)
                    inst["sync_info"] = si
                out.append(inst)
            bb["instructions"] = out
    return bir


def _install_waitfix(nc):
    orig = nc.to_json_bytes

    def patched(self):
        return _json.dumps(_patch_bir(_json.loads(orig()))).encode()

    nc.to_json_bytes = _MethodType(patched, nc)
    return nc


# ----------------------------------------------------------- host weights --


def _prep_consts(p):
    """Fold reference weights into device layouts. p: dict of np arrays."""
    f16 = lambda x: np.ascontiguousarray(x, dtype=np.float16)
    f32 = lambda x: np.ascontiguousarray(x, dtype=np.float32)
    C = np.eye(64, dtype=np.float64) - 1.0 / 64.0

    c = {}
    c["W1c"] = f16(p["W1"].astype(np.float64) @ C)  # [128, 64]
    c["W2c"] = f16(p["W2"].astype(np.float64) @ C)  # [96, 64]
    # per-partition bias vectors (2-stacked for the paired t-block layout)
    c["b1s"] = f32(np.tile(np.asarray(p["b1"], np.float64) @ C, 2).reshape(-1, 1))
    c["b2s"] = f32(np.tile(np.asarray(p["b2"], np.float64) @ C, 2).reshape(-1, 1))
    c["be1r"] = f32(np.tile(p["be1"], 2).reshape(-1, 1))
    c["be2r"] = f32(np.tile(p["be2"], 2).reshape(-1, 1))
    # g folded into the rstd replicate matmuls: grep [2,128], col m is
    # [g[m],0] for m<64 else [0,g[m-64]]
    def grep(g):
        m = np.zeros((2, 128), np.float64)
        m[0, 0:64] = g
        m[1, 64:128] = g
        return f16(m)

    c["grep1"] = grep(np.asarray(p["g1"], np.float64))
    c["grep2"] = grep(np.asarray(p["g2"], np.float64))
    c["ones_red"] = f16(
        np.block(
            [[np.ones((64, 1)), np.zeros((64, 1))], [np.zeros((64, 1)), np.ones((64, 1))]]
        )
    )  # [128, 2]

    # Gate weights, all-tanh folding. Reference W rows = [x(0:64); h(64:128)],
    # cols = [i j f o]. Our lhsT rows = XH rows = [htilde(0:64); x(64:128)],
    # htilde = 2h -> h-rows get an extra 0.5. Sigmoid gates (i,f,o) computed
    # as tanh(z/2): their cols get 0.5; j stays tanh(z).
    def fold_gates(Wref, bref, pre):
        Wref = np.asarray(Wref, np.float64)
        bref = np.asarray(bref, np.float64)
        Wx, Wh = Wref[0:64], Wref[64:128]
        sl = dict(i=slice(0, 64), j=slice(64, 128), f=slice(128, 192), o=slice(192, 256))

        def cols(g, cs):
            return np.concatenate([Wh[:, sl[g]] * 0.5 * cs, Wx[:, sl[g]] * cs], 0)

        c[f"Wfi{pre}"] = f16(np.concatenate([cols("f", 0.5), cols("i", 0.5)], 1))
        c[f"Woj{pre}"] = f16(np.concatenate([cols("o", 0.5), cols("j", 1.0)], 1))
        c[f"bfi{pre}"] = f32(
            np.concatenate([(bref[sl["f"]] + 1.0) * 0.5, bref[sl["i"]] * 0.5]).reshape(-1, 1)
        )
        c[f"boj{pre}"] = f32(
            np.concatenate([bref[sl["o"]] * 0.5, bref[sl["j"]]]).reshape(-1, 1)
        )

    fold_gates(p["Wf"], p["bf"], "f")
    fold_gates(p["Wb"], p["bb"], "b")

    c["whead"] = f16(0.5 * np.stack([np.asarray(p["wx"], np.float64),
                                     np.asarray(p["wp"], np.float64)], 1))  # [64, 2]

    # Head params; bw direction is time-reversed relative to our bw scan
    # state order, handled by reversing the per-t parameters.
    c["CT"] = f16(np.eye(64) - 1.0 / 64.0)
    c["ones64"] = f16(np.ones((64, 1)))
    c["W3f"] = f16(p["W3"])
    c["W3b"] = f16(np.asarray(p["W3"])[::-1, ::-1])
    c["b3f"] = f32(np.asarray(p["b3"]).reshape(64, 1))
    c["b3b"] = f32(np.asarray(p["b3"])[::-1].reshape(64, 1))
    gp = np.asarray(p["gp"], np.float64)
    c["grephf"] = f16(gp.reshape(1, 64))
    c["grephb"] = f16(gp[::-1].reshape(1, 64))
    c["bephf"] = f32(np.asarray(p["bep"]).reshape(64, 1))
    c["bephb"] = f32(np.asarray(p["bep"])[::-1].reshape(64, 1))
    c["bx"] = float(np.asarray(p["bx"]))
    return c


_CONST_F16 = (
    "W1c", "W2c", "grep1", "grep2", "ones_red",
    "Wfif", "Wojf", "Wfib", "Wojb", "whead",
    "CT", "ones64", "W3f", "W3b", "grephf", "grephb",
)
_CONST_F32 = (
    "b1s", "b2s", "be1r", "be2r", "bfif", "bojf", "bfib", "bojb",
    "b3f", "b3b", "bephf", "bephb",
)


# ------------------------------------------------------------ bass program --


def _build(consts):
    nc = bass.Bass()
    obsT = nc.declare_dram_parameter("obsT", [128, NT], F16, isOutput=False)
    actT = nc.declare_dram_parameter("actT", [32, NT], F16, isOutput=False)

    cin = {}
    for name in _CONST_F16:
        cin[name] = nc.declare_dram_parameter(name, list(consts[name].shape), F16, isOutput=False)
    for name in _CONST_F32:
        cin[name] = nc.declare_dram_parameter(name, list(consts[name].shape), F32, isOutput=False)

    ov = nc.declare_dram_parameter("ov", [2, BC], F32, isOutput=True)
    xs_dram = nc.dram_tensor("xs_stash", [4, NT], F16)

    with tile.TileContext(nc) as tc:
        with (
            tc.tile_pool(name="singles", bufs=1) as sing,
            tc.tile_pool(name="obs_p", bufs=2) as obs_p,
            tc.tile_pool(name="work", bufs=2) as work,
            tc.tile_pool(name="small", bufs=1) as small,
        ):
            # ---- load constants ----
            ct = {}
            for name, dram in cin.items():
                a = consts[name]
                ct[name] = sing.tile(list(a.shape), dram.dtype, name=f"ct_{name}", tag=f"ct_{name}")
                nc.sync.dma_start(out=ct[name], in_=dram[:, :])

            # ---- LSTM state arrays (65 column-blocks of BC) ----
            XHf = sing.tile([128, 65 * BC], F16)  # col t: [htf_{t-1}; x_t]
            XHb = sing.tile([128, 65 * BC], F16)  # col 64-t: [htb_{t-1}; x_{63-t}]
            ctil = sing.tile([64, 2 * BC], F16)  # ctilde, per chain [64, 2*HB]
            epst = sing.tile([128, 1], F32)
            nc.vector.memset(epst, EPS)
            nc.vector.memset(XHf[0:64, 0:BC], 0.0)
            nc.vector.memset(XHb[0:64, 64 * BC : 65 * BC], 0.0)
            nc.vector.memset(ctil, 0.0)

            # ================= Phase T: trunk =================
            with (
                tc.tile_pool(name="t_pv", bufs=2, space="PSUM") as ps_pv,
                tc.tile_pool(name="t_ss", bufs=2, space="PSUM") as ps_ss,
                tc.tile_pool(name="t_rp", bufs=2, space="PSUM") as ps_rp,
            ):
                for j in range(32):
                    ta, tb = j, 63 - j
                    ob2 = obs_p.tile([128, 2 * BC], F16, tag="obs")
                    nc.sync.dma_start(out=ob2, in_=obsT[:, j * 2 * BC : (j + 1) * 2 * BC])
                    xab = obs_p.tile([96, 2 * BC], F16, tag="xab")
                    nc.sync.dma_start(
                        out=xab[64:96, :], in_=actT[:, j * 2 * BC : (j + 1) * 2 * BC]
                    )

                    # ---- layer 1 ----
                    pv = ps_pv.tile([128, BC], F32, tag="pv")
                    nc.tensor.matmul(pv[0:64, :], ct["W1c"], ob2[:, 0:BC],
                                     start=True, stop=True, tile_position=(0, 0))
                    nc.tensor.matmul(pv[64:128, :], ct["W1c"], ob2[:, BC : 2 * BC],
                                     start=True, stop=True, tile_position=(0, 64))
                    v_s = work.tile([128, BC], F16, tag="v_s")
                    nc.scalar.activation(v_s, pv, AF.Identity, bias=ct["b1s"][:, 0:1])
                    sq = work.tile([128, BC], F16, tag="sq")
                    nc.vector.tensor_mul(sq, v_s, v_s)
                    pss = ps_ss.tile([2, BC], F32, tag="pss")
                    nc.tensor.matmul(pss, ct["ones_red"], sq, start=True, stop=True)
                    # rsqrt(v) = exp(-0.5*ln(v)) -- ACT Rsqrt is blocked, and
                    # ln/exp share one activation table with the head softmax
                    lnv = small.tile([2, BC], F32, tag="lnv")
                    nc.scalar.activation(lnv, pss, AF.Ln, bias=epst[0:2, 0:1], scale=1.0 / 64.0)
                    rstd = small.tile([2, BC], F16, tag="rstd")
                    nc.scalar.activation(rstd, lnv, AF.Exp, scale=-0.5)
                    prep = ps_rp.tile([128, BC], F32, tag="prep")
                    nc.tensor.matmul(prep, ct["grep1"], rstd, start=True, stop=True)
                    xn = work.tile([128, BC], F16, tag="xn")
                    nc.vector.tensor_mul(xn, v_s, prep)
                    # relu+beta into the two [96,2BC] halves (layer-2 input)
                    nc.vector.tensor_scalar(
                        xab[0:64, 0:BC], xn[0:64, :], ct["be1r"][0:64, 0:1], 0.0,
                        ALU.add, ALU.max,
                    )
                    nc.vector.tensor_scalar(
                        xab[0:64, BC : 2 * BC], xn[64:128, :], ct["be1r"][64:128, 0:1], 0.0,
                        ALU.add, ALU.max,
                    )

                    # ---- layer 2 ----
                    pv2 = ps_pv.tile([128, BC], F32, tag="pv")
                    nc.tensor.matmul(pv2[0:64, :], ct["W2c"], xab[:, 0:BC],
                                     start=True, stop=True, tile_position=(0, 0))
                    nc.tensor.matmul(pv2[64:128, :], ct["W2c"], xab[:, BC : 2 * BC],
                                     start=True, stop=True, tile_position=(0, 64))
                    v_s2 = work.tile([128, BC], F16, tag="v_s")
                    nc.scalar.activation(v_s2, pv2, AF.Identity, bias=ct["b2s"][:, 0:1])
                    sq2 = work.tile([128, BC], F16, tag="sq")
                    nc.vector.tensor_mul(sq2, v_s2, v_s2)
                    pss2 = ps_ss.tile([2, BC], F32, tag="pss")
                    nc.tensor.matmul(pss2, ct["ones_red"], sq2, start=True, stop=True)
                    lnv2 = small.tile([2, BC], F32, tag="lnv")
                    nc.scalar.activation(lnv2, pss2, AF.Ln, bias=epst[0:2, 0:1], scale=1.0 / 64.0)
                    rstd2 = small.tile([2, BC], F16, tag="rstd")
                    nc.scalar.activation(rstd2, lnv2, AF.Exp, scale=-0.5)
                    prep2 = ps_rp.tile([128, BC], F32, tag="prep")
                    nc.tensor.matmul(prep2, ct["grep2"], rstd2, start=True, stop=True)
                    xn2 = work.tile([128, BC], F16, tag="xn")
                    nc.vector.tensor_mul(xn2, v_s2, prep2)
                    # x_t into both direction arrays:
                    #   fw: XHf col t rows 64:128; bw: XHb col (t+1) rows 64:128
                    nc.vector.tensor_scalar(
                        XHf[64:128, ta * BC : (ta + 1) * BC], xn2[0:64, :],
                        ct["be2r"][0:64, 0:1], 0.0, ALU.add, ALU.max,
                    )
                    nc.vector.tensor_scalar(
                        XHf[64:128, tb * BC : (tb + 1) * BC], xn2[64:128, :],
                        ct["be2r"][64:128, 0:1], 0.0, ALU.add, ALU.max,
                    )
                    nc.gpsimd.tensor_copy(
                        XHb[64:128, (ta + 1) * BC : (ta + 2) * BC],
                        XHf[64:128, ta * BC : (ta + 1) * BC],
                    )
                    nc.gpsimd.tensor_copy(
                        XHb[64:128, (tb + 1) * BC : (tb + 2) * BC],
                        XHf[64:128, tb * BC : (tb + 1) * BC],
                    )

            tc.strict_bb_all_engine_barrier()

            # ================= Phase L: LSTM =================
            # per chain: T1 (f|i), T2 (o|j) psum [128, 2*HB] (fw cols 0:HB,
            # bw cols HB:2HB); head-dot psum [2, 2*BC] per dir (2 steps).
            bias_same = bool(
                np.array_equal(consts["bfif"], consts["bfib"])
                and np.array_equal(consts["bojf"], consts["bojb"])
            )
            with (
                tc.tile_pool(name="l_t1", bufs=1, space="PSUM") as ps_t1,
                tc.tile_pool(name="l_t2", bufs=1, space="PSUM") as ps_t2,
                tc.tile_pool(name="l_ph", bufs=1, space="PSUM") as ps_ph,
            ):
                phf = ps_ph.tile([2, 2 * BC], F32, tag="phf")
                phb = ps_ph.tile([2, 2 * BC], F32, tag="phb")
                stg_f = [sing.tile([2, 2 * BC], F16, name=f"stgf{i}", tag=f"stgf{i}") for i in range(2)]
                stg_b = [sing.tile([2, 2 * BC], F16, name=f"stgb{i}", tag=f"stgb{i}") for i in range(2)]

                TI = [sing.tile([128, 2 * HB], F16, name=f"TI{ch}", tag=f"TI{ch}") for ch in range(2)]
                TO = [sing.tile([128, 2 * HB], F16, name=f"TO{ch}", tag=f"TO{ch}") for ch in range(2)]
                Bt = [sing.tile([64, 2 * HB], F16, name=f"Bt{ch}", tag=f"Bt{ch}") for ch in range(2)]
                At = [sing.tile([64, 2 * HB], F16, name=f"At{ch}", tag=f"At{ch}") for ch in range(2)]
                tcl = [sing.tile([64, 2 * HB], F16, name=f"tc{ch}", tag=f"tc{ch}") for ch in range(2)]

                def lstep(t):
                    fcol = t * BC  # fw rhs col block
                    bcol = (64 - t) * BC  # bw rhs col block
                    for chh in range(2):
                        co = chh * HB
                        t1 = ps_t1.tile([128, 2 * HB], F32, tag=f"t1c{chh}")
                        nc.tensor.matmul(t1[:, 0:HB], ct["Wfif"],
                                         XHf[:, fcol + co : fcol + co + HB],
                                         start=True, stop=True)
                        nc.tensor.matmul(t1[:, HB : 2 * HB], ct["Wfib"],
                                         XHb[:, bcol + co : bcol + co + HB],
                                         start=True, stop=True)
                        t2 = ps_t2.tile([128, 2 * HB], F32, tag=f"t2c{chh}")
                        nc.tensor.matmul(t2[:, 0:HB], ct["Wojf"],
                                         XHf[:, fcol + co : fcol + co + HB],
                                         start=True, stop=True)
                        nc.tensor.matmul(t2[:, HB : 2 * HB], ct["Wojb"],
                                         XHb[:, bcol + co : bcol + co + HB],
                                         start=True, stop=True)
                        ti, to = TI[chh], TO[chh]
                        # bias is per-partition; when the fw/bw bias vectors
                        # are identical (zero-bias nets) one [128, 2HB] tanh
                        # covers both direction halves of the psum tile.
                        if bias_same:
                            nc.scalar.activation(ti, t1, AF.Tanh, bias=ct["bfif"][:, 0:1])
                            nc.scalar.activation(to, t2, AF.Tanh, bias=ct["bojf"][:, 0:1])
                        else:
                            nc.scalar.activation(ti[:, 0:HB], t1[:, 0:HB], AF.Tanh,
                                                 bias=ct["bfif"][:, 0:1])
                            nc.scalar.activation(ti[:, HB : 2 * HB], t1[:, HB : 2 * HB],
                                                 AF.Tanh, bias=ct["bfib"][:, 0:1])
                            nc.scalar.activation(to[:, 0:HB], t2[:, 0:HB], AF.Tanh,
                                                 bias=ct["bojf"][:, 0:1])
                            nc.scalar.activation(to[:, HB : 2 * HB], t2[:, HB : 2 * HB],
                                                 AF.Tanh, bias=ct["bojb"][:, 0:1])
                        bt, at, tcc = Bt[chh], At[chh], tcl[chh]
                        cti = ctil[:, chh * 2 * HB : (chh + 1) * 2 * HB]
                        # B = (ti+1)*tj ; A = (tf+1)*ctilde ; ct' = A/2 + B
                        nc.vector.scalar_tensor_tensor(
                            bt, ti[64:128, :], 1.0, to[64:128, :],
                            ALU.add, ALU.mult,
                        )
                        nc.vector.scalar_tensor_tensor(
                            at, ti[0:64, :], 1.0, cti, ALU.add, ALU.mult,
                        )
                        nc.vector.scalar_tensor_tensor(
                            cti, at, 0.5, bt, ALU.mult, ALU.add,
                        )
                        nc.scalar.activation(tcc, cti, AF.Tanh, scale=0.5)
                        # htilde = (to+1)*tanh(c) -> next col of XH
                        nc.vector.scalar_tensor_tensor(
                            XHf[0:64, fcol + BC + co : fcol + BC + co + HB],
                            to[0:64, 0:HB], 1.0, tcc[:, 0:HB], ALU.add, ALU.mult,
                        )
                        nc.vector.scalar_tensor_tensor(
                            XHb[0:64, bcol - BC + co : bcol - BC + co + HB],
                            to[0:64, HB : 2 * HB], 1.0, tcc[:, HB : 2 * HB],
                            ALU.add, ALU.mult,
                        )

                    # head dots for state h_{t} (in XH col t+1), full batch
                    s = t  # state index
                    slot = (s % 2) * BC
                    nc.tensor.matmul(phf[0:2, slot : slot + BC], ct["whead"],
                                     XHf[0:64, (t + 1) * BC : (t + 2) * BC],
                                     start=True, stop=True, tile_position=(0, 0))
                    nc.tensor.matmul(phb[0:2, slot : slot + BC], ct["whead"],
                                     XHb[0:64, (63 - t) * BC : (64 - t) * BC],
                                     start=True, stop=True, tile_position=(0, 0))
                    if s % 2 == 1:
                        dcol = (s - 1) * BC
                        sf = stg_f[(s // 2) % 2]
                        sb = stg_b[(s // 2) % 2]
                        nc.vector.tensor_copy(sf, phf)
                        nc.vector.tensor_copy(sb, phb)
                        nc.sync.dma_start(out=xs_dram[0:2, dcol : dcol + 2 * BC], in_=sf)
                        nc.sync.dma_start(out=xs_dram[2:4, dcol : dcol + 2 * BC], in_=sb)

                for t in range(T):
                    lstep(t)

            tc.strict_bb_all_engine_barrier()

            # ================= Phase H: head =================
            with tc.tile_pool(name="head_ps", bufs=1, space="PSUM") as ps_h:
                pn_d = []
                xs_d = []
                for d, (w3, b3, gph, beph) in enumerate(
                    (("W3f", "b3f", "grephf", "bephf"), ("W3b", "b3b", "grephb", "bephb"))
                ):
                    xsT = work.tile([64, BC], F16, tag="hxs")
                    psT = work.tile([64, BC], F16, tag="hps")
                    nc.sync.dma_start(
                        out=xsT,
                        in_=xs_dram[2 * d : 2 * d + 1, :].rearrange("o (t b) -> (o t) b", b=BC),
                    )
                    nc.sync.dma_start(
                        out=psT,
                        in_=xs_dram[2 * d + 1 : 2 * d + 2, :].rearrange("o (t b) -> (o t) b", b=BC),
                    )
                    pc = ps_h.tile([64, BC], F32, tag="hpc")
                    nc.tensor.matmul(pc, ct["CT"], psT, start=True, stop=True)
                    hsq = work.tile([64, BC], F16, tag="hsq")
                    nc.scalar.activation(hsq, pc, AF.Square)
                    hss = ps_h.tile([1, BC], F32, tag="hss")
                    nc.tensor.matmul(hss, ct["ones64"], hsq, start=True, stop=True)
                    hlnv = small.tile([1, BC], F32, tag="hlnv")
                    nc.scalar.activation(hlnv, hss, AF.Ln, bias=epst[0:1, 0:1], scale=1.0 / 64.0)
                    hrst = small.tile([1, BC], F16, tag="hrst")
                    nc.scalar.activation(hrst, hlnv, AF.Exp, scale=-0.5)
                    hrep = ps_h.tile([64, BC], F32, tag="hrep")
                    nc.tensor.matmul(hrep, ct[gph], hrst, start=True, stop=True)
                    hcs = work.tile([64, BC], F16, tag="hcs")
                    nc.scalar.activation(hcs, pc, AF.Identity)
                    ht1 = work.tile([64, BC], F16, tag="ht1")
                    nc.vector.tensor_mul(ht1, hcs, hrep)
                    pn = work.tile([64, BC], F16, tag="hpn")
                    nc.vector.tensor_scalar(
                        pn, ht1, ct[beph][:, 0:1], 0.0, ALU.add, ALU.max
                    )
                    pn_d.append((pn, w3, b3))
                    xs_d.append(xsT)

                ovs0 = sing.tile([1, BC], F32)
                ovs1 = sing.tile([1, BC], F32)
                for d, ((pn, w3, b3), xsT) in enumerate(zip(pn_d, xs_d)):
                    pl = ps_h.tile([64, BC], F32, tag="hpl")
                    nc.tensor.matmul(pl, ct[w3], pn, start=True, stop=True)
                    he16 = work.tile([64, BC], F16, tag="he16")
                    nc.scalar.activation(he16, pl, AF.Exp, bias=ct[b3][:, 0:1])
                    hse = ps_h.tile([1, BC], F32, tag="hse")
                    nc.tensor.matmul(hse, ct["ones64"], he16, start=True, stop=True)
                    hex16 = work.tile([64, BC], F16, tag="hex16")
                    nc.vector.tensor_mul(hex16, he16, xsT)
                    hnum = ps_h.tile([1, BC], F32, tag="hnum")
                    nc.tensor.matmul(hnum, ct["ones64"], hex16, start=True, stop=True)
                    # 1/sum via exp(-ln(sum)); no DVE reciprocal
                    hlns = small.tile([1, BC], F32, tag="hlns")
                    nc.scalar.activation(hlns, hse, AF.Ln)
                    hrse = small.tile([1, BC], F32, tag="hrse")
                    nc.scalar.activation(hrse, hlns, AF.Exp, scale=-1.0)
                    hov = small.tile([1, BC], F32, tag="hov")
                    nc.vector.tensor_mul(hov, hnum, hrse)
                    nc.vector.tensor_scalar(
                        ovs0 if d == 0 else ovs1, hov, float(consts["bx"]), None, ALU.add
                    )
                nc.sync.dma_start(out=ov[0:1, :], in_=ovs0)
                nc.sync.dma_start(out=ov[1:2, :], in_=ovs1)

    return nc


_CACHE = {}
LAST_RESULTS = None


def kernel(**inputs):
    obs = np.asarray(inputs["obs"])
    action = np.asarray(inputs["action"])
    consts = _prep_consts(inputs)

    key = "nc"
    if key not in _CACHE:
        _CACHE[key] = _install_waitfix(_build(consts))
    nc = _CACHE[key]

    const_feed = {k: consts[k] for k in _CONST_F16 + _CONST_F32}

    ordv = np.array(ORD)
    in_maps = []
    for c in range(NCORES):
        sl = slice(c * BC * T, (c + 1) * BC * T)
        # [BC, T, F] -> [F, T(ORD), BC] in fp16
        ob = obs[sl].reshape(BC, T, OBS).transpose(2, 1, 0)[:, ordv, :]
        ac = action[sl].reshape(BC, T, ACTD).transpose(2, 1, 0)[:, ordv, :]
        m = {
            "obsT": np.ascontiguousarray(ob.reshape(OBS, NT), dtype=np.float16),
            "actT": np.ascontiguousarray(ac.reshape(ACTD, NT), dtype=np.float16),
        }
        m.update(const_feed)
        in_maps.append(m)

    global LAST_RESULTS
    LAST_RESULTS = run_bass_kernel_spmd(nc, in_maps, list(range(NCORES)))
    res = LAST_RESULTS.results

    out = np.empty(2 * B, dtype=np.float32)
    for c in range(NCORES):
        ovc = res[c]["ov"]
        out[c * BC : (c + 1) * BC] = ovc[0]
        out[B + c * BC : B + (c + 1) * BC] = ovc[1]
    return out
